# revision 17
# baseline (speedup 1.0000x reference)
"""Trainium2 Bass kernel for nn_DisplacementField (tri-plane nearest-neighbor
embedding lookup).

Reference semantics: for each of N=1M points with coords (x,y,z) and time
t01 in [0,1):
    t  = 2*t01 - 1;  p = -pts / 1.6
    ix   = round(((t   + 1) * 0.5) * 127)            in [0,127]
    iy_a = clip(round(((p_a + 1) * 0.5) * 511), 0, 511)
    feat = prod_a plane_a[:, iy_a, ix]               -> [N, 32]
feature_A/feature_B both == feat except (possibly) the last row (the
reference shifts only data[-1]); that row and the scalar cond select are
fixed on the host in exact f32 numpy.

Device strategy (8 cores, data-parallel over N):
  - planes repacked host-side to [H*W/2, 64] f32 "pair tables": row
    r = iy*64 + (ix>>1) holds the 128B vectors for ix even|odd. Row index
    fits int16 (<= 32767) as required by the SWDGE dma_gather ISA, whose
    elem_size must be a 256B multiple.
  - The gpsimd engine executes SWDGE dma_gather ucode strictly serially
    (one instruction at a time, ~2.1-2.8ns/idx), and the gather ucode
    arbitrates with DVE for the shared SBUF port pair (exclusive
    per-instruction lock), so the kernel keeps the 48-gather stream
    dense and keeps contending work off its path:
      * ALL per-point index arithmetic (bit-exact IEEE f32, round-half-
        even via the +2^23 magic trick) runs up front on whole-J
        [128, 992] tiles (plane order 2,1,0 so plane-0 rows finish
        last, right when the first gather needs them); no index math
        (which would grab the shared port) overlaps the gather stream.
      * PE selection matmuls fold [128,62] f32 row indices into the
        gather ISA's wrapped+replicated [128, 8*62] int16 layout
        (index j of the gather list lives at partition j%16 (all 8
        groups), slot j//16); the (b,c)-interleaving f32->i16 cast out
        of PSUM runs on the ACT engine (own queue + dedicated SBUF
        ports) so a product op blocked on a gather-DMA semaphore can
        never starve the idx-tile supply.
      * dma_gather fetches 256B/point/plane on queues RR 0-3 into a
        9-deep buffer pool (3 chunks of lookahead absorbs the ~25us
        DMA-drain lag behind descriptor generation).
      * chunk k's products + parity select (DVE, in place) are emitted
        only after chunk k+1's gathers (software pipelining), and the
        compact copy feeding the contiguous store runs on ACT.
  - out point order is partition-minor (point i -> partition i%128,
    slot i//128) as dictated by dma_gather; the host permutes shards
    to/from that order (part of sharding/unsharding).
All device arithmetic is bit-identical to the f32 reference chain.
"""

import numpy as np

N = 1_000_000
FEAT = 32
RES_H = 512
RES_W = 128
BOUNDS = 1.6
TIME_STEP = 1.0 / (2.0 * RES_W)
NCORES = 8

# per-core layout: 128 partitions x J points, processed in NCHUNK chunks of
# C slots; wrap PSUM slab per (chunk,plane) is [128, 8, C] f32 = 1 PSUM bank.
# Within each chunk, cols [0, C8) hold plane-0 "staircase" groups: 8
# consecutive slots whose plane-0 rows are exactly base..base+7, fetched by
# ONE 2KB gather descriptor (the whole kernel is DMA-descriptor-RATE bound,
# so 8 points per descriptor on plane 0 cuts wall time ~22%). Cols [C8, C)
# are per-point singles. The host orders points (quantile-sharded and
# sorted by plane-0 row) to realize the staircases.
J = 1008
C = 63
C8 = 48                    # staircase cols per chunk (6 groups of 8)
NST = C8 // 8              # staircase groups per chunk per partition
C1 = C - C8                # single cols per chunk
NCHUNK = J // C            # 16
NPC = 128 * J              # 129,024 points per core
NPAD = NPC * NCORES        # 1,032,192
STAIR_CAP = NCHUNK * NST * 128   # staircases per core (12,288)
SING_CAP = NCHUNK * C1 * 128     # single slots per core (30,720)

MAGIC = 8388608.0          # 2^23: x+MAGIC-MAGIC == round-half-even(x), 0<=x<2^22

_CACHE = {}


def _build_nc():
    from concourse import bass, bacc, mybir
    import concourse.tile as tile

    f32 = mybir.dt.float32
    bf16 = mybir.dt.bfloat16
    i16 = mybir.dt.int16
    i32 = mybir.dt.int32
    Alu = mybir.AluOpType

    def raw_dma_gather(g, out_ap, in_ap, idxs_ap, num_idxs, elem_size,
                       elem_step, queue_num):
        # nc.gpsimd.dma_gather minus its elem_size%256B assert (the 256B
        # restriction only applies to transpose mode; the non-transpose
        # ucode handles any elem size — verified on HW). elem/step in
        # elements of in_ap dtype; stride must be a 256B multiple.
        assert (elem_step * mybir.dt.size(in_ap.dtype)) % 256 == 0
        return g.add_instruction(
            mybir.InstDMAGatherAnt(
                name=g.bass.get_next_instruction_name(),
                ins=[
                    *g.lower_ap_dma(in_ap, for_custom_bir_dma=True),
                    g.lower_ap(idxs_ap),
                    g.lower_val_access(g.to_reg(num_idxs)),
                ],
                outs=[g.lower_ap(out_ap)],
                transpose=False,
                num_idxs=num_idxs,
                elem_size=elem_size,
                stride_bytes_256=(elem_step * mybir.dt.size(in_ap.dtype)) // 256,
                gen_mode=0,
                single_packet=False,
                queue_num=queue_num,
                sbuf_tokens_per_rank=0,
                sbuf_free_dim_per_rank=0,
                sbuf_free_dim_pad_per_rank=0,
                sbuf_byte_offset=0,
            )
        )

    # 4 SWDGE queues: spreads descriptor rings over 4x16 DMA-engine rings so
    # the serially-executing gather ucode never stalls on ring space
    nc = bacc.Bacc("TRN2", target_bir_lowering=False, num_swdge_queues=4)
    pn = nc.dram_tensor("pnorm", [3, 128, J], f32, kind="ExternalInput")
    tm = nc.dram_tensor("time_in", [128, J], f32, kind="ExternalInput")
    sel_in = nc.dram_tensor("sel_in", [128, 8 * 128], f32, kind="ExternalInput")
    tabs = [
        nc.dram_tensor(f"tab{a}", [RES_H * RES_W // 2, 2 * FEAT], f32,
                       kind="ExternalInput")
        for a in range(3)
    ]
    feat = nc.dram_tensor("feat", [128, J, FEAT], f32, kind="ExternalOutput")

    NIDX = 128 * C

    with tile.TileContext(nc) as tc:
        with (
            tc.tile_pool(name="const", bufs=1) as cp,
            tc.tile_pool(name="tmp", bufs=2) as tp,
            tc.tile_pool(name="w", bufs=4) as wp,
            tc.tile_pool(name="g", bufs=8) as gp,
            tc.tile_pool(name="fc", bufs=1) as fp,
            tc.tile_pool(name="ps", bufs=1, space="PSUM") as pp,
        ):
            sel = cp.tile([128, 8 * 128], f32)
            nc.sync.dma_start(out=sel[:], in_=sel_in[:])
            t0 = cp.tile([128, J], f32)
            nc.scalar.dma_start(out=t0[:], in_=tm[:])
            # per-plane contiguous coord tiles, loaded in chain order (the
            # plane-2 index chain runs first) so the first v-chain never
            # waits on the whole coords transfer
            pcoord = []
            for a in range(3):
                pc = cp.tile([128, J], f32, tag=f"pc_{a}")
                pcoord.append(pc)
            for a in (2, 1, 0):
                nc.sync.dma_start(out=pcoord[a][:], in_=pn[a])

            # ---- whole-J index math.
            # time path: fx = rhe(((t+1)*0.5)*127), t = 2*t01-1.  Fused
            # tensor_scalar op pairs are chosen so the result is identical
            # whether or not the intermediate rounds to f32 (2nd op is an
            # exact pow2 scale / exact add / min/max / Sterbenz subtract).
            t2 = tp.tile([128, J], f32, tag="t")
            nc.vector.tensor_scalar(
                out=t2[:], in0=t0[:], scalar1=2.0, scalar2=1.0,
                op0=Alu.mult, op1=Alu.subtract)
            u1 = tp.tile([128, J], f32, tag="t")
            nc.vector.tensor_scalar(
                out=u1[:], in0=t2[:], scalar1=1.0, scalar2=0.5,
                op0=Alu.add, op1=Alu.mult)
            u2 = tp.tile([128, J], f32, tag="t")
            nc.vector.tensor_scalar_mul(u2[:], u1[:], 127.0)
            mx = tp.tile([128, J], f32, tag="t")
            nc.vector.tensor_scalar_add(mx[:], u2[:], MAGIC)
            fx = cp.tile([128, J], f32)
            nc.vector.tensor_scalar_sub(fx[:], mx[:], MAGIC)
            # qr = ix>>1 = rhe(fx*0.5 - 0.25)  (both ops exact)
            q1 = tp.tile([128, J], f32, tag="t")
            nc.vector.tensor_scalar(
                out=q1[:], in0=fx[:], scalar1=0.5, scalar2=-0.25,
                op0=Alu.mult, op1=Alu.add)
            mq = tp.tile([128, J], f32, tag="t")
            nc.vector.tensor_scalar_add(mq[:], q1[:], MAGIC)
            qr = cp.tile([128, J], f32)
            nc.vector.tensor_scalar_sub(qr[:], mq[:], MAGIC)
            rowfs = [None, None, None]
            for a in (2, 1, 0):
                # iy: v = ((p+1)*0.5)*511, clip, rhe; row = iy*64 + qr
                v1 = tp.tile([128, J], f32, tag="t")
                nc.vector.tensor_scalar(
                    out=v1[:], in0=pcoord[a][:], scalar1=1.0, scalar2=0.5,
                    op0=Alu.add, op1=Alu.mult)
                v2 = tp.tile([128, J], f32, tag="t")
                nc.vector.tensor_scalar(
                    out=v2[:], in0=v1[:], scalar1=511.0, scalar2=0.0,
                    op0=Alu.mult, op1=Alu.max)
                m3 = tp.tile([128, J], f32, tag="t")
                nc.vector.tensor_scalar(
                    out=m3[:], in0=v2[:], scalar1=511.0, scalar2=MAGIC,
                    op0=Alu.min, op1=Alu.add)
                f64 = tp.tile([128, J], f32, tag="t")
                nc.vector.tensor_scalar(
                    out=f64[:], in0=m3[:], scalar1=MAGIC, scalar2=64.0,
                    op0=Alu.subtract, op1=Alu.mult)
                rowf = cp.tile([128, J], f32, tag=f"rowf_{a}")
                nc.vector.tensor_tensor(
                    out=rowf[:], in0=f64[:], in1=qr[:], op=Alu.add)
                rowfs[a] = rowf

            # parity bit = fx - 2*qr (exact); off the rowf critical path --
            # first consumer is consume(0), well after the gathers start
            tb = tp.tile([128, J], f32, tag="t")
            nc.vector.tensor_scalar_mul(tb[:], qr[:], 2.0)
            bitf = tp.tile([128, J], f32, tag="t")
            nc.vector.tensor_tensor(
                out=bitf[:], in0=fx[:], in1=tb[:], op=Alu.subtract)
            bit = cp.tile([128, J], i32)
            nc.scalar.copy(out=bit[:], in_=bitf[:])

            def consume(k, gs, halves=1):
                for h in range(halves):
                    hs = slice(h * (C // halves), (h + 1) * (C // halves))
                    sl = slice(k * C + h * (C // halves),
                               k * C + (h + 1) * (C // halves))
                    consume_part(sl, hs, gs)

            def consume_part(sl, hs, gs):
                # product on 64-wide pairs, then select the 128B half by the
                # shared ix-parity bit (in place); compact on the Activation
                # engine (dedicated SBUF ports -- keeps the shared DVE/GpSimd
                # port pair free for the gather ucode) so the store is one
                # contiguous DMA
                n = sl.stop - sl.start
                nc.vector.tensor_tensor(
                    out=gs[0][:, hs, :], in0=gs[0][:, hs, :],
                    in1=gs[1][:, hs, :], op=Alu.mult)
                nc.vector.tensor_tensor(
                    out=gs[0][:, hs, :], in0=gs[0][:, hs, :],
                    in1=gs[2][:, hs, :], op=Alu.mult)
                pred = bit[:, sl][:, :, None].to_broadcast([128, n, FEAT])
                nc.vector.copy_predicated(
                    out=gs[0][:, hs, 0:FEAT], mask=pred,
                    data=gs[0][:, hs, FEAT:2 * FEAT])
                fc = fp.tile([128, C, FEAT], f32, tag="fc")
                nc.scalar.copy(out=fc[:, 0:n, :], in_=gs[0][:, hs, 0:FEAT])
                nc.sync.dma_start(out=feat[:, sl, :], in_=fc[:, 0:n, :])

            def make_wrapped(rhs_ap, ncols, tag):
                # fold [128,ncols] row indices into wrapped [128, 8*ncols]
                # int16: 8 selection matmuls (psum[:, b, :] = rows
                # b*16..b*16+15 of rhs replicated to all 8 partition groups),
                # then one strided ACT copy interleaving (b, c) -> slot c*8+b.
                ps = pp.tile([128, 8, ncols], f32, tag=f"ps_{tag}")
                for b in range(8):
                    nc.tensor.matmul(
                        out=ps[:, b, :],
                        lhsT=sel[:, b * 128:(b + 1) * 128],
                        rhs=rhs_ap,
                        start=True, stop=True)
                wrapped = wp.tile([128, 8 * ncols], i16, tag=f"w_{tag}")
                wr_view = bass.AP(
                    wrapped.tensor, wrapped[:].offset,
                    [wrapped[:].ap[0], (1, 8), (8, ncols)])
                nc.scalar.copy(out=wr_view, in_=ps[:])
                return wrapped

            pending = None
            for k in range(NCHUNK):
                sl = slice(k * C, (k + 1) * C)
                # plane 0: staircase stream (cols 0..C8) + singles (C8..C)
                g0 = gp.tile([128, C, 2 * FEAT], f32, tag="g")
                g1 = gp.tile([128, C, 2 * FEAT], f32, tag="g")
                g2 = gp.tile([128, C, 2 * FEAT], f32, tag="g")
                gs = [g0, g1, g2]
                r0 = rowfs[0]

                def emit_stair():
                    stair_rhs = bass.AP(
                        r0.tensor, r0[:, k * C:k * C + 1].offset,
                        [r0[:].ap[0], (8, NST)])
                    w8 = make_wrapped(stair_rhs, NST, "s8")
                    g0v8 = bass.AP(
                        g0.tensor, g0[:].offset,
                        [g0[:].ap[0], (8 * 2 * FEAT, NST), (1, 8 * 2 * FEAT)])
                    raw_dma_gather(
                        nc.gpsimd, g0v8, tabs[0][:], w8[:],
                        num_idxs=NST * 128, elem_size=8 * 2 * FEAT,
                        elem_step=2 * FEAT, queue_num=0)

                def emit_single():
                    w1 = make_wrapped(r0[:, k * C + C8:(k + 1) * C], C1, "s1")
                    g0v1 = bass.AP(
                        g0.tensor, g0[:, C8:C8 + 1, :].offset,
                        [g0[:].ap[0], (2 * FEAT, C1), (1, 2 * FEAT)])
                    raw_dma_gather(
                        nc.gpsimd, g0v1, tabs[0][:], w1[:],
                        num_idxs=C1 * 128, elem_size=2 * FEAT,
                        elem_step=2 * FEAT, queue_num=0)

                def emit_plane(a, lo, hi, part):
                    # half-chunk plane gathers: smaller SWDGE ring-await
                    # quanta pipeline much better than one 8064-desc gather
                    def f():
                        w = make_wrapped(
                            rowfs[a][:, k * C + lo:k * C + hi], hi - lo,
                            f"p{a}{part}")
                        g = gs[a]
                        gv = bass.AP(
                            g.tensor, g[:, lo:lo + 1, :].offset,
                            [g[:].ap[0], (2 * FEAT, hi - lo), (1, 2 * FEAT)])
                        raw_dma_gather(
                            nc.gpsimd, gv, tabs[a][:], w[:],
                            num_idxs=(hi - lo) * 128, elem_size=2 * FEAT,
                            elem_step=2 * FEAT, queue_num=0)
                    return f

                # rotate emission order chunk-to-chunk: the Tile DMASW-sem
                # fixup below re-derives queue = scheduled-order % 4, so
                # rotation cycles every stream across all 4 queues, and the
                # heavy plane gathers are split in half so each queue-ring
                # await covers at most ~4k descriptors.
                emits = [emit_plane(1, 0, 32, "a"), emit_stair,
                         emit_plane(2, 0, 32, "a"), emit_plane(1, 32, C, "b"),
                         emit_single, emit_plane(2, 32, C, "b")]
                for i in range(6):
                    emits[(i + k) % 6]()

                # consume the previous chunk only now: keeps this chunk's
                # gather stream ahead of the product ops in every engine queue
                if pending is not None:
                    consume(k - 1, pending)
                pending = gs
            consume(NCHUNK - 1, pending, halves=3)

    # Tile assigns DMASW completion sems round-robin in *scheduled* order,
    # and the SWDGE ucode requires each DMASW sem to be driven by a single
    # queue. Re-derive queue_num from the assigned sem so sem i belongs to
    # queue i%4 always.
    import re
    for blk in nc.main_func.blocks:
        for ins in blk.instructions:
            if isinstance(ins, mybir.InstDMAGatherAnt) and ins.sync_info:
                for u in ins.sync_info.on_update:
                    m = re.match(r"DMASW(\d+)_", getattr(u, "ant_name", "") or "")
                    if m:
                        ins.queue_num = int(m.group(1)) % 4
    nc.finalize()
    return nc


def _get_nc():
    if "nc" not in _CACHE:
        _CACHE["nc"] = _build_nc()
    return _CACHE["nc"]


def _make_sel():
    # sel_in[p, b*128 + p'] = 1 iff p == b*16 + (p' % 16)
    sel = np.zeros((128, 8, 128), dtype=np.float32)
    for b in range(8):
        for pp_ in range(128):
            sel[b * 16 + (pp_ % 16), b, pp_] = 1.0
    return sel.reshape(128, 8 * 128)


def _pack_tables(planes):
    # [F,H,W] -> [H*W, F] -> pair view [H*W/2, 2F]; row iy*64+(ix>>1)
    return [
        np.ascontiguousarray(
            np.asarray(p, dtype=np.float32).transpose(1, 2, 0)
        ).reshape(RES_H * RES_W // 2, 2 * FEAT)
        for p in planes
    ]


def _rows_plane0(pnorm0, t01):
    """Exact-f32 replication of the device plane-0 pair-row index chain."""
    one, half = np.float32(1.0), np.float32(0.5)
    t2 = (t01 * np.float32(2.0)) - one
    u2 = ((t2 + one) * half) * np.float32(127.0)
    ix = np.clip(np.round(u2).astype(np.int32), 0, 127)
    v = ((pnorm0 + one) * half) * np.float32(511.0)
    iy = np.clip(np.round(v).astype(np.int32), 0, 511)
    return iy * 64 + (ix >> 1)


def _layout_points(pnorm, t01):
    """Global device point order: quantile-shard by plane-0 pair-row, then
    per core pack points into staircase-8 groups (8 slots whose plane-0 rows
    are exactly base..base+7 -> one 2KB gather descriptor) and singles.

    Returns perm[NPAD]: perm[i] = original point id at padded slot i
    (slot i = core i//NPC, within-core slot (i%NPC) in partition-minor
    order: partition (i%NPC)%128, column (i%NPC)//128), or -1 for padding.
    """
    rows = _rows_plane0(pnorm[:, 0], t01)
    order = np.argsort(rows, kind="stable")
    perm = np.full(NPAD, -1, dtype=np.int64)

    n_real = N // NCORES          # 125,000 real points per core
    for c in range(NCORES):
        ids = order[c * n_real:(c + 1) * n_real]
        rr = rows[ids]            # sorted ascending (stable argsort)
        # per-row span boundaries in the sorted list
        starts = np.searchsorted(rr, np.arange(32769))
        ptr = starts[:-1].copy()  # next unused point per row
        avail = np.diff(starts).astype(np.int64)

        # greedy staircase walk: batches of m staircases at base i
        batches = []
        nst = 0
        i = 0
        av = avail.copy()
        while i <= 32760 and nst < STAIR_CAP:
            w = av[i:i + 8]
            m = int(w.min())
            if m > 0:
                m = min(m, STAIR_CAP - nst)
                av[i:i + 8] -= m
                batches.append((i, nst, m))
                nst += m
            else:
                i += 1

        # slot grids for this core
        grid = np.full((128, J), -1, dtype=np.int64)
        # assign staircase points: stair s -> partition s%128, chunk/group
        # kt = s//128 (k = kt//NST, t = kt%NST), col k*C + 8t + o
        for base, s0, m in batches:
            s = np.arange(s0, s0 + m)
            p = s % 128
            kt = s // 128
            col0 = (kt // NST) * C + 8 * (kt % NST)
            for o in range(8):
                r = base + o
                take = ids[ptr[r]:ptr[r] + m]
                ptr[r] += m
                grid[p, col0 + o] = take

        # leftovers + padding -> singles (cols k*C+C8 .. k*C+C-1)
        left = []
        for r in np.nonzero(starts[1:] - ptr > 0)[0]:
            left.append(ids[ptr[r]:starts[r + 1]])
        left = np.concatenate(left) if left else np.empty(0, dtype=np.int64)
        assert left.size <= SING_CAP, (c, left.size)
        sl = np.arange(left.size)
        p = sl % 128
        r = sl // 128
        col = (r // C1) * C + C8 + (r % C1)
        grid[p, col] = left
        # fill any unused staircase region slots too (shouldn't happen when
        # stair cap is reached, but keep it total): remaining -1 slots stay
        # padding.
        perm[c * NPC:(c + 1) * NPC] = grid.T.ravel()  # slot (p,j) = j*128+p
    return perm


def _host_feat_row(prow, trow, planes):
    """Exact f32 replication of the reference gather/product for one point."""
    one = np.float32(1.0)
    half = np.float32(0.5)
    acc = np.float32(1.0)
    for a, plane in enumerate(planes):
        u = ((trow + one) * half) * np.float32(RES_W - 1)
        ix = int(np.clip(np.round(u).astype(np.int32), 0, RES_W - 1))
        v = ((prow[a] + one) * half) * np.float32(RES_H - 1)
        iy = int(np.clip(np.round(v).astype(np.int32), 0, RES_H - 1))
        acc = (acc * plane[:, iy, ix].astype(np.float32)).astype(np.float32)
    return acc


def _make_in_maps(pnorm, t01, planes):
    perm = _layout_points(pnorm, t01)
    valid = perm >= 0
    pn_pad = np.zeros((NPAD, 3), dtype=np.float32)
    pn_pad[valid] = pnorm[perm[valid]]
    t_pad = np.zeros(NPAD, dtype=np.float32)
    t_pad[valid] = t01[perm[valid]]

    tabs = _pack_tables(planes)
    sel = _make_sel()

    in_maps = []
    for c in range(NCORES):
        s = slice(c * NPC, (c + 1) * NPC)
        # device point order is partition-minor: slot i -> (i%128, i//128);
        # coords per-plane contiguous [3, 128, J]
        pn_dev = np.ascontiguousarray(
            pn_pad[s].reshape(J, 128, 3).transpose(2, 1, 0))
        t_dev = np.ascontiguousarray(t_pad[s].reshape(J, 128).T)
        in_maps.append({
            "pnorm": pn_dev,
            "time_in": t_dev,
            "sel_in": sel,
            "tab0": tabs[0],
            "tab1": tabs[1],
            "tab2": tabs[2],
        })
    return in_maps, perm


def _device_feat(pnorm, t01, planes, trace=False, **kw):
    """Run the 8-core device kernel; returns (feat[:N], BassKernelResults)."""
    from concourse.bass_utils import run_bass_kernel_spmd

    in_maps, perm = _make_in_maps(pnorm, t01, planes)
    nc = _get_nc()
    res = run_bass_kernel_spmd(nc, in_maps, list(range(NCORES)), trace=trace, **kw)
    feat_lin = np.empty((NPAD, FEAT), dtype=np.float32)
    for c in range(NCORES):
        # undo partition-minor order
        feat_lin[c * NPC:(c + 1) * NPC] = (
            res.results[c]["feat"].transpose(1, 0, 2).reshape(NPC, FEAT))
    feat = np.empty((N, FEAT), dtype=np.float32)
    valid = perm >= 0
    feat[perm[valid]] = feat_lin[valid]
    return feat, res


def kernel(pts, time, plane0, plane1, plane2):
    pts = np.asarray(pts, dtype=np.float32)
    time = np.asarray(time, dtype=np.float32)
    planes = tuple(np.asarray(p, dtype=np.float32) for p in (plane0, plane1, plane2))

    # host: exact f32 normalization (single IEEE divide, matches XLA bitwise)
    pnorm = np.divide(np.negative(pts), np.float32(BOUNDS), dtype=np.float32)
    t01 = time[:, 0]

    feat_orig, _ = _device_feat(pnorm, t01, planes)

    # host fix-up for the reference's last-row shift quirk (exact f32)
    ts32 = np.float32(TIME_STEP)
    p_last = pnorm[-1].copy()
    t_last = np.float32(time[-1, 0] * np.float32(2.0) - np.float32(1.0))
    p_shift = (p_last - ts32).astype(np.float32)
    t_shift = np.float32(t_last - ts32)
    shift_row = _host_feat_row(p_shift, t_shift, planes)

    cond = bool(p_last[0] + ts32 > np.float32(1.0))

    feature_A = feat_orig
    feature_B = feat_orig.copy()
    if cond:
        feature_A = feature_A.copy()
        feature_A[-1] = shift_row
    else:
        feature_B[-1] = shift_row
    return feature_A, feature_B



# revision 18
# speedup vs baseline: 1.2311x; 1.2311x over previous
"""Trainium2 Bass kernel for nn_DisplacementField (tri-plane nearest-neighbor
embedding lookup).

Reference semantics: for each of N=1M points with coords (x,y,z) and time
t01 in [0,1):
    t  = 2*t01 - 1;  p = -pts / 1.6
    ix   = round(((t   + 1) * 0.5) * 127)            in [0,127]
    iy_a = clip(round(((p_a + 1) * 0.5) * 511), 0, 511)
    feat = prod_a plane_a[:, iy_a, ix]               -> [N, 32]
feature_A/feature_B both == feat except (possibly) the last row (the
reference shifts only data[-1]); that row and the scalar cond select are
fixed on the host in exact f32 numpy.

Device strategy (8 cores, data-parallel over N):
  - planes repacked host-side to [H*W/2, 64] f32 "pair tables": row
    r = iy*64 + (ix>>1) holds the 128B vectors for ix even|odd. Row index
    fits int16 (<= 32767) as required by the SWDGE dma_gather ISA, whose
    elem_size must be a 256B multiple.
  - The gpsimd engine executes SWDGE dma_gather ucode strictly serially
    (one instruction at a time, ~2.1-2.8ns/idx), and the gather ucode
    arbitrates with DVE for the shared SBUF port pair (exclusive
    per-instruction lock), so the kernel keeps the 48-gather stream
    dense and keeps contending work off its path:
      * ALL per-point index arithmetic (bit-exact IEEE f32, round-half-
        even via the +2^23 magic trick) runs up front on whole-J
        [128, 992] tiles (plane order 2,1,0 so plane-0 rows finish
        last, right when the first gather needs them); no index math
        (which would grab the shared port) overlaps the gather stream.
      * PE selection matmuls fold [128,62] f32 row indices into the
        gather ISA's wrapped+replicated [128, 8*62] int16 layout
        (index j of the gather list lives at partition j%16 (all 8
        groups), slot j//16); the (b,c)-interleaving f32->i16 cast out
        of PSUM runs on the ACT engine (own queue + dedicated SBUF
        ports) so a product op blocked on a gather-DMA semaphore can
        never starve the idx-tile supply.
      * dma_gather fetches 256B/point/plane on queues RR 0-3 into a
        9-deep buffer pool (3 chunks of lookahead absorbs the ~25us
        DMA-drain lag behind descriptor generation).
      * chunk k's products + parity select (DVE, in place) are emitted
        only after chunk k+1's gathers (software pipelining), and the
        compact copy feeding the contiguous store runs on ACT.
  - out point order is partition-minor (point i -> partition i%128,
    slot i//128) as dictated by dma_gather; the host permutes shards
    to/from that order (part of sharding/unsharding).
All device arithmetic is bit-identical to the f32 reference chain.
"""

import numpy as np
import ml_dtypes

N = 1_000_000
FEAT = 32
RES_H = 512
RES_W = 128
BOUNDS = 1.6
TIME_STEP = 1.0 / (2.0 * RES_W)
NCORES = 8

# per-core layout: 128 partitions x J points, processed in NCHUNK chunks of
# C slots; wrap PSUM slab per (chunk,plane) is [128, 8, C] f32 = 1 PSUM bank
J = 992
C = 62
NCHUNK = J // C            # 16
NPC = 128 * J              # 126,976 points per core
NPAD = NPC * NCORES        # 1,015,808

MAGIC = 8388608.0          # 2^23: x+MAGIC-MAGIC == round-half-even(x), 0<=x<2^22

_CACHE = {}


def _build_nc():
    from concourse import bass, bacc, mybir
    import concourse.tile as tile

    f32 = mybir.dt.float32
    bf16 = mybir.dt.bfloat16
    i16 = mybir.dt.int16
    i32 = mybir.dt.int32
    Alu = mybir.AluOpType

    def raw_dma_gather(g, out_ap, in_ap, idxs_ap, num_idxs, elem_size,
                       elem_step, queue_num):
        # nc.gpsimd.dma_gather minus its elem_size%256B assert (that
        # restriction only applies to transpose mode; the non-transpose
        # ucode handles any elem size -- verified on HW). elem/step are in
        # elements of in_ap dtype; the row stride must be a 256B multiple.
        assert (elem_step * mybir.dt.size(in_ap.dtype)) % 256 == 0
        return g.add_instruction(
            mybir.InstDMAGatherAnt(
                name=g.bass.get_next_instruction_name(),
                ins=[
                    *g.lower_ap_dma(in_ap, for_custom_bir_dma=True),
                    g.lower_ap(idxs_ap),
                    g.lower_val_access(g.to_reg(num_idxs)),
                ],
                outs=[g.lower_ap(out_ap)],
                transpose=False,
                num_idxs=num_idxs,
                elem_size=elem_size,
                stride_bytes_256=(elem_step * mybir.dt.size(in_ap.dtype)) // 256,
                gen_mode=0,
                single_packet=False,
                queue_num=queue_num,
                sbuf_tokens_per_rank=0,
                sbuf_free_dim_per_rank=0,
                sbuf_free_dim_pad_per_rank=0,
                sbuf_byte_offset=0,
            )
        )

    # 4 SWDGE queues: spreads descriptor rings over 4x16 DMA-engine rings so
    # the serially-executing gather ucode never stalls on ring space
    nc = bacc.Bacc("TRN2", target_bir_lowering=False, num_swdge_queues=4)
    pn = nc.dram_tensor("pnorm", [3, 128, J], f32, kind="ExternalInput")
    tm = nc.dram_tensor("time_in", [128, J], f32, kind="ExternalInput")
    sel_in = nc.dram_tensor("sel_in", [128, 8 * 128], f32, kind="ExternalInput")
    # bf16 pair tables padded to a 256B row stride (SWDGE row stride must
    # be a 256B multiple); each gather desc fetches only the 128B payload
    # half, halving gather DMA bytes vs f32.
    tabs = [
        nc.dram_tensor(f"tab{a}", [RES_H * RES_W // 2, 4 * FEAT], bf16,
                       kind="ExternalInput")
        for a in range(3)
    ]
    feat = nc.dram_tensor("feat", [128, J, FEAT], f32, kind="ExternalOutput")

    NIDX = 128 * C

    with tile.TileContext(nc) as tc:
        with (
            tc.tile_pool(name="const", bufs=1) as cp,
            tc.tile_pool(name="tmp", bufs=2) as tp,
            tc.tile_pool(name="w", bufs=9) as wp,
            tc.tile_pool(name="g", bufs=9) as gp,
            tc.tile_pool(name="fc", bufs=1) as fp,
            tc.tile_pool(name="ps", bufs=8, space="PSUM") as pp,
        ):
            sel = cp.tile([128, 8 * 128], f32)
            nc.sync.dma_start(out=sel[:], in_=sel_in[:])
            t0 = cp.tile([128, J], f32)
            nc.scalar.dma_start(out=t0[:], in_=tm[:])
            # per-plane contiguous coord tiles, loaded in chain order (the
            # plane-2 index chain runs first) so the first v-chain never
            # waits on the whole coords transfer
            pcoord = []
            for a in range(3):
                pc = cp.tile([128, J], f32, tag=f"pc_{a}")
                pcoord.append(pc)
            for a in (2, 1, 0):
                nc.sync.dma_start(out=pcoord[a][:], in_=pn[a])

            # ---- whole-J index math.
            # time path: fx = rhe(((t+1)*0.5)*127), t = 2*t01-1.  Fused
            # tensor_scalar op pairs are chosen so the result is identical
            # whether or not the intermediate rounds to f32 (2nd op is an
            # exact pow2 scale / exact add / min/max / Sterbenz subtract).
            t2 = tp.tile([128, J], f32, tag="t")
            nc.vector.tensor_scalar(
                out=t2[:], in0=t0[:], scalar1=2.0, scalar2=1.0,
                op0=Alu.mult, op1=Alu.subtract)
            u1 = tp.tile([128, J], f32, tag="t")
            nc.vector.tensor_scalar(
                out=u1[:], in0=t2[:], scalar1=1.0, scalar2=0.5,
                op0=Alu.add, op1=Alu.mult)
            u2 = tp.tile([128, J], f32, tag="t")
            nc.vector.tensor_scalar_mul(u2[:], u1[:], 127.0)
            mx = tp.tile([128, J], f32, tag="t")
            nc.vector.tensor_scalar_add(mx[:], u2[:], MAGIC)
            fx = cp.tile([128, J], f32)
            nc.vector.tensor_scalar_sub(fx[:], mx[:], MAGIC)
            # qr = ix>>1 = rhe(fx*0.5 - 0.25)  (both ops exact)
            q1 = tp.tile([128, J], f32, tag="t")
            nc.vector.tensor_scalar(
                out=q1[:], in0=fx[:], scalar1=0.5, scalar2=-0.25,
                op0=Alu.mult, op1=Alu.add)
            mq = tp.tile([128, J], f32, tag="t")
            nc.vector.tensor_scalar_add(mq[:], q1[:], MAGIC)
            qr = cp.tile([128, J], f32)
            nc.vector.tensor_scalar_sub(qr[:], mq[:], MAGIC)
            rowfs = [None, None, None]
            for a in (2, 1, 0):
                # iy: v = ((p+1)*0.5)*511, clip, rhe; row = iy*64 + qr
                v1 = tp.tile([128, J], f32, tag="t")
                nc.vector.tensor_scalar(
                    out=v1[:], in0=pcoord[a][:], scalar1=1.0, scalar2=0.5,
                    op0=Alu.add, op1=Alu.mult)
                v2 = tp.tile([128, J], f32, tag="t")
                nc.vector.tensor_scalar(
                    out=v2[:], in0=v1[:], scalar1=511.0, scalar2=0.0,
                    op0=Alu.mult, op1=Alu.max)
                m3 = tp.tile([128, J], f32, tag="t")
                nc.vector.tensor_scalar(
                    out=m3[:], in0=v2[:], scalar1=511.0, scalar2=MAGIC,
                    op0=Alu.min, op1=Alu.add)
                f64 = tp.tile([128, J], f32, tag="t")
                nc.vector.tensor_scalar(
                    out=f64[:], in0=m3[:], scalar1=MAGIC, scalar2=64.0,
                    op0=Alu.subtract, op1=Alu.mult)
                rowf = cp.tile([128, J], f32, tag=f"rowf_{a}")
                nc.vector.tensor_tensor(
                    out=rowf[:], in0=f64[:], in1=qr[:], op=Alu.add)
                rowfs[a] = rowf

            # parity bit = fx - 2*qr (exact); off the rowf critical path --
            # first consumer is consume(0), well after the gathers start
            tb = tp.tile([128, J], f32, tag="t")
            nc.vector.tensor_scalar_mul(tb[:], qr[:], 2.0)
            bitf = tp.tile([128, J], f32, tag="t")
            nc.vector.tensor_tensor(
                out=bitf[:], in0=fx[:], in1=tb[:], op=Alu.subtract)
            bit = cp.tile([128, J], i32)
            nc.scalar.copy(out=bit[:], in_=bitf[:])

            def consume(k, gs, halves=1):
                for h in range(halves):
                    hs = slice(h * (C // halves), (h + 1) * (C // halves))
                    sl = slice(k * C + h * (C // halves),
                               k * C + (h + 1) * (C // halves))
                    consume_part(sl, hs, gs)

            def consume_part(sl, hs, gs):
                # product on 64-wide pairs, then select the 128B half by the
                # shared ix-parity bit (in place); compact on the Activation
                # engine (dedicated SBUF ports -- keeps the shared DVE/GpSimd
                # port pair free for the gather ucode) so the store is one
                # contiguous DMA
                n = sl.stop - sl.start
                nc.vector.tensor_tensor(
                    out=gs[0][:, hs, :], in0=gs[0][:, hs, :],
                    in1=gs[1][:, hs, :], op=Alu.mult)
                nc.vector.tensor_tensor(
                    out=gs[0][:, hs, :], in0=gs[0][:, hs, :],
                    in1=gs[2][:, hs, :], op=Alu.mult)
                pred = bit[:, sl][:, :, None].to_broadcast([128, n, FEAT])
                nc.vector.copy_predicated(
                    out=gs[0][:, hs, 0:FEAT], mask=pred,
                    data=gs[0][:, hs, FEAT:2 * FEAT])
                fc = fp.tile([128, C, FEAT], f32, tag="fc")
                nc.scalar.copy(out=fc[:, 0:n, :], in_=gs[0][:, hs, 0:FEAT])
                nc.sync.dma_start(out=feat[:, sl, :], in_=fc[:, 0:n, :])

            pending = None
            for k in range(NCHUNK):
                sl = slice(k * C, (k + 1) * C)
                gs = []
                for a in range(3):
                    # fold [128,C] row indices into wrapped [128, 8C] int16:
                    # 8 selection matmuls (psum[:, b, :] = rows b*16..b*16+15
                    # of rowf replicated to all 8 partition groups), then one
                    # strided copy interleaving (b, c) -> slot c*8+b.
                    ps = pp.tile([128, 8, C], f32, tag="ps")
                    for b in range(8):
                        nc.tensor.matmul(
                            out=ps[:, b, :],
                            lhsT=sel[:, b * 128:(b + 1) * 128],
                            rhs=rowfs[a][:, sl],
                            start=True, stop=True)
                    wrapped = wp.tile([128, 8 * C], i16, tag="w")
                    wr_view = bass.AP(
                        wrapped.tensor, wrapped[:].offset,
                        [wrapped[:].ap[0], (1, 8), (8, C)])
                    # interleave+cast on ACT: its own engine queue + dedicated
                    # SBUF ports, so a product op blocked on a gather-DMA sem
                    # can never starve the idx-tile supply
                    nc.scalar.copy(out=wr_view, in_=ps[:])

                    g = gp.tile([128, C, 2 * FEAT], bf16, tag="g")
                    raw_dma_gather(
                        nc.gpsimd, g[:], tabs[a][:, 0:2 * FEAT], wrapped[:],
                        num_idxs=NIDX, elem_size=2 * FEAT,
                        elem_step=4 * FEAT, queue_num=(k * 3 + a) % 4,
                    )
                    gs.append(g)

                # consume the previous chunk only now: keeps this chunk's
                # gather stream ahead of the product ops in every engine queue
                if pending is not None:
                    consume(k - 1, pending)
                pending = gs
            consume(NCHUNK - 1, pending, halves=2)

    # Tile assigns DMASW completion sems round-robin in *scheduled* order,
    # and the SWDGE ucode requires each DMASW sem to be driven by a single
    # queue. Re-derive queue_num from the assigned sem so sem i belongs to
    # queue i%4 always.
    import re
    for blk in nc.main_func.blocks:
        for ins in blk.instructions:
            if isinstance(ins, mybir.InstDMAGatherAnt) and ins.sync_info:
                for u in ins.sync_info.on_update:
                    m = re.match(r"DMASW(\d+)_", getattr(u, "ant_name", "") or "")
                    if m:
                        ins.queue_num = int(m.group(1)) % 4
    nc.finalize()
    return nc


def _get_nc():
    if "nc" not in _CACHE:
        _CACHE["nc"] = _build_nc()
    return _CACHE["nc"]


def _make_sel():
    # sel_in[p, b*128 + p'] = 1 iff p == b*16 + (p' % 16)
    sel = np.zeros((128, 8, 128), dtype=np.float32)
    for b in range(8):
        for pp_ in range(128):
            sel[b * 16 + (pp_ % 16), b, pp_] = 1.0
    return sel.reshape(128, 8 * 128)


def _pack_tables(planes):
    # [F,H,W] -> [H*W, F] -> pair view [H*W/2, 2F] bf16, padded to a 256B
    # row stride; row iy*64+(ix>>1), payload in the first 2F columns
    out = []
    for p in planes:
        pairs = np.ascontiguousarray(
            np.asarray(p, dtype=np.float32).transpose(1, 2, 0)
        ).reshape(RES_H * RES_W // 2, 2 * FEAT).astype(ml_dtypes.bfloat16)
        tab = np.zeros((RES_H * RES_W // 2, 4 * FEAT), dtype=ml_dtypes.bfloat16)
        tab[:, :2 * FEAT] = pairs
        out.append(tab)
    return out


def _host_feat_row(prow, trow, planes):
    """Exact f32 replication of the reference gather/product for one point."""
    one = np.float32(1.0)
    half = np.float32(0.5)
    acc = np.float32(1.0)
    for a, plane in enumerate(planes):
        u = ((trow + one) * half) * np.float32(RES_W - 1)
        ix = int(np.clip(np.round(u).astype(np.int32), 0, RES_W - 1))
        v = ((prow[a] + one) * half) * np.float32(RES_H - 1)
        iy = int(np.clip(np.round(v).astype(np.int32), 0, RES_H - 1))
        acc = (acc * plane[:, iy, ix].astype(np.float32)).astype(np.float32)
    return acc


def _make_in_maps(pnorm, t01, planes):
    pn_pad = np.zeros((NPAD, 3), dtype=np.float32)
    pn_pad[:N] = pnorm
    t_pad = np.zeros(NPAD, dtype=np.float32)
    t_pad[:N] = t01

    tabs = _pack_tables(planes)
    sel = _make_sel()

    in_maps = []
    for c in range(NCORES):
        s = slice(c * NPC, (c + 1) * NPC)
        # device point order is partition-minor: point i -> (i%128, i//128);
        # coords per-plane contiguous [3, 128, J]
        pn_dev = np.ascontiguousarray(
            pn_pad[s].reshape(J, 128, 3).transpose(2, 1, 0))
        t_dev = np.ascontiguousarray(t_pad[s].reshape(J, 128).T)
        in_maps.append({
            "pnorm": pn_dev,
            "time_in": t_dev,
            "sel_in": sel,
            "tab0": tabs[0],
            "tab1": tabs[1],
            "tab2": tabs[2],
        })
    return in_maps


def _device_feat(pnorm, t01, planes, trace=False, **kw):
    """Run the 8-core device kernel; returns (feat[:N], BassKernelResults)."""
    from concourse.bass_utils import run_bass_kernel_spmd

    in_maps = _make_in_maps(pnorm, t01, planes)
    nc = _get_nc()
    res = run_bass_kernel_spmd(nc, in_maps, list(range(NCORES)), trace=trace, **kw)
    feat = np.empty((NPAD, FEAT), dtype=np.float32)
    for c in range(NCORES):
        # undo partition-minor order
        feat[c * NPC:(c + 1) * NPC] = (
            res.results[c]["feat"].transpose(1, 0, 2).reshape(NPC, FEAT))
    return feat[:N], res


def kernel(pts, time, plane0, plane1, plane2):
    pts = np.asarray(pts, dtype=np.float32)
    time = np.asarray(time, dtype=np.float32)
    planes = tuple(np.asarray(p, dtype=np.float32) for p in (plane0, plane1, plane2))

    # host: exact f32 normalization (single IEEE divide, matches XLA bitwise)
    pnorm = np.divide(np.negative(pts), np.float32(BOUNDS), dtype=np.float32)
    t01 = time[:, 0]

    feat_orig, _ = _device_feat(pnorm, t01, planes)

    # host fix-up for the reference's last-row shift quirk (exact f32)
    ts32 = np.float32(TIME_STEP)
    p_last = pnorm[-1].copy()
    t_last = np.float32(time[-1, 0] * np.float32(2.0) - np.float32(1.0))
    p_shift = (p_last - ts32).astype(np.float32)
    t_shift = np.float32(t_last - ts32)
    shift_row = _host_feat_row(p_shift, t_shift, planes)

    cond = bool(p_last[0] + ts32 > np.float32(1.0))

    feature_A = feat_orig
    feature_B = feat_orig.copy()
    if cond:
        feature_A = feature_A.copy()
        feature_A[-1] = shift_row
    else:
        feature_B[-1] = shift_row
    return feature_A, feature_B



# revision 22
# speedup vs baseline: 1.2363x; 1.0042x over previous
"""Trainium2 Bass kernel for nn_DisplacementField (tri-plane nearest-neighbor
embedding lookup).

Reference semantics: for each of N=1M points with coords (x,y,z) and time
t01 in [0,1):
    t  = 2*t01 - 1;  p = -pts / 1.6
    ix   = round(((t   + 1) * 0.5) * 127)            in [0,127]
    iy_a = clip(round(((p_a + 1) * 0.5) * 511), 0, 511)
    feat = prod_a plane_a[:, iy_a, ix]               -> [N, 32]
feature_A/feature_B both == feat except (possibly) the last row (the
reference shifts only data[-1]); that row and the scalar cond select are
fixed on the host in exact f32 numpy.

Device strategy (8 cores, data-parallel over N):
  - planes repacked host-side to [H*W/2, 128] bf16 "pair tables" (payload
    in the first 64 cols, padded to a 256B row stride): row r = iy*64 +
    (ix>>1) holds the 64B bf16 vectors for ix even|odd. Row index fits
    int16 (<= 32767) as required by the SWDGE dma_gather ISA. The ISA's
    elem_size%256B rule only binds transpose mode, so a raw-built
    InstDMAGatherAnt fetches just the 128B payload at the 256B stride,
    halving gather HBM traffic vs f32 (verified exact in CoreSim; products
    run bf16 in/out, final compact casts to f32; max rel err 1.63e-2 on
    the fixed seed-0 dataset, inside the 2e-2 gate).
  - The gpsimd engine executes SWDGE dma_gather ucode strictly serially
    (one instruction at a time, ~2.1-2.8ns/idx), and the gather ucode
    arbitrates with DVE for the shared SBUF port pair (exclusive
    per-instruction lock), so the kernel keeps the 48-gather stream
    dense and keeps contending work off its path:
      * ALL per-point index arithmetic (bit-exact IEEE f32, round-half-
        even via the +2^23 magic trick) runs up front on whole-J
        [128, 992] tiles (plane order 2,1,0 so plane-0 rows finish
        last, right when the first gather needs them); no index math
        (which would grab the shared port) overlaps the gather stream.
      * PE selection matmuls fold [128,62] f32 row indices into the
        gather ISA's wrapped+replicated [128, 8*62] int16 layout
        (index j of the gather list lives at partition j%16 (all 8
        groups), slot j//16); the (b,c)-interleaving f32->i16 cast out
        of PSUM runs on the ACT engine (own queue + dedicated SBUF
        ports) so a product op blocked on a gather-DMA semaphore can
        never starve the idx-tile supply.
      * dma_gather fetches 256B/point/plane on queues RR 0-3 into a
        9-deep buffer pool (3 chunks of lookahead absorbs the ~25us
        DMA-drain lag behind descriptor generation).
      * chunk k's products + parity select (DVE, in place) are emitted
        only after chunk k+1's gathers (software pipelining), and the
        compact copy feeding the contiguous store runs on ACT.
  - out point order is partition-minor (point i -> partition i%128,
    slot i//128) as dictated by dma_gather; the host permutes shards
    to/from that order (part of sharding/unsharding).
Index arithmetic is bit-identical to the f32 reference chain; table
values and products are bf16-rounded as described above.

Perf notes from this tuning session (trace-verified): the kernel is bound
by SWDGE descriptor DRAIN, not gpsimd desc-gen (unblocked gathers run at
0.42ns/idx; the ucode stalls in-instruction awaiting ring space while the
16 DMA engines process the small descriptors at ~2.2ns/desc aggregate,
nearly independent of payload size 128B..256B). Fewer descriptors is the
only real lever: an experiment packing 8 host-sorted points per 2KB
descriptor (plane-0 "staircases") cut descriptors 22% but big descriptors
drain ~3x worse per byte per ring and per-instruction overheads grew, so
it measured slower (1.01ms); this layout measured 833us vs 860us for the
f32 baseline.
"""

import numpy as np
import ml_dtypes

N = 1_000_000
FEAT = 32
RES_H = 512
RES_W = 128
BOUNDS = 1.6
TIME_STEP = 1.0 / (2.0 * RES_W)
NCORES = 8

# per-core layout: 128 partitions x J points, processed in NCHUNK chunks of
# C slots; wrap PSUM slab per (chunk,plane) is [128, 8, C] f32 = 1 PSUM bank
J = 992
C = 62
NCHUNK = J // C            # 16
NPC = 128 * J              # 126,976 points per core
NPAD = NPC * NCORES        # 1,015,808

MAGIC = 8388608.0          # 2^23: x+MAGIC-MAGIC == round-half-even(x), 0<=x<2^22

_CACHE = {}


def _build_nc():
    from concourse import bass, bacc, mybir
    import concourse.tile as tile

    f32 = mybir.dt.float32
    bf16 = mybir.dt.bfloat16
    i16 = mybir.dt.int16
    i32 = mybir.dt.int32
    Alu = mybir.AluOpType

    def raw_dma_gather(g, out_ap, in_ap, idxs_ap, num_idxs, elem_size,
                       elem_step, queue_num):
        # nc.gpsimd.dma_gather minus its elem_size%256B assert (that
        # restriction only applies to transpose mode; the non-transpose
        # ucode handles any elem size -- verified on HW). elem/step are in
        # elements of in_ap dtype; the row stride must be a 256B multiple.
        assert (elem_step * mybir.dt.size(in_ap.dtype)) % 256 == 0
        return g.add_instruction(
            mybir.InstDMAGatherAnt(
                name=g.bass.get_next_instruction_name(),
                ins=[
                    *g.lower_ap_dma(in_ap, for_custom_bir_dma=True),
                    g.lower_ap(idxs_ap),
                    g.lower_val_access(g.to_reg(num_idxs)),
                ],
                outs=[g.lower_ap(out_ap)],
                transpose=False,
                num_idxs=num_idxs,
                elem_size=elem_size,
                stride_bytes_256=(elem_step * mybir.dt.size(in_ap.dtype)) // 256,
                gen_mode=0,
                single_packet=False,
                queue_num=queue_num,
                sbuf_tokens_per_rank=0,
                sbuf_free_dim_per_rank=0,
                sbuf_free_dim_pad_per_rank=0,
                sbuf_byte_offset=0,
            )
        )

    # 4 SWDGE queues: spreads descriptor rings over 4x16 DMA-engine rings so
    # the serially-executing gather ucode never stalls on ring space
    nc = bacc.Bacc("TRN2", target_bir_lowering=False, num_swdge_queues=4)
    pn = nc.dram_tensor("pnorm", [3, 128, J], f32, kind="ExternalInput")
    tm = nc.dram_tensor("time_in", [128, J], f32, kind="ExternalInput")
    sel_in = nc.dram_tensor("sel_in", [128, 8 * 128], f32, kind="ExternalInput")
    # bf16 pair tables padded to a 256B row stride (SWDGE row stride must
    # be a 256B multiple); each gather desc fetches only the 128B payload
    # half, halving gather DMA bytes vs f32.
    tabs = [
        nc.dram_tensor(f"tab{a}", [RES_H * RES_W // 2, 4 * FEAT], bf16,
                       kind="ExternalInput")
        for a in range(3)
    ]
    # output stays bf16: the product chain is already bf16-valued, so
    # storing bf16 is numerically identical and halves store traffic
    feat = nc.dram_tensor("feat", [128, J, FEAT], bf16, kind="ExternalOutput")

    NIDX = 128 * C

    with tile.TileContext(nc) as tc:
        with (
            tc.tile_pool(name="const", bufs=1) as cp,
            tc.tile_pool(name="tmp", bufs=2) as tp,
            tc.tile_pool(name="w", bufs=9) as wp,
            tc.tile_pool(name="g", bufs=9) as gp,
            tc.tile_pool(name="fc", bufs=1) as fp,
            tc.tile_pool(name="ps", bufs=8, space="PSUM") as pp,
        ):
            sel = cp.tile([128, 8 * 128], f32)
            nc.sync.dma_start(out=sel[:], in_=sel_in[:])
            t0 = cp.tile([128, J], f32)
            nc.scalar.dma_start(out=t0[:], in_=tm[:])
            # per-plane contiguous coord tiles, loaded in chain order (the
            # plane-2 index chain runs first) so the first v-chain never
            # waits on the whole coords transfer
            pcoord = []
            for a in range(3):
                pc = cp.tile([128, J], f32, tag=f"pc_{a}")
                pcoord.append(pc)
            for a in (2, 1, 0):
                nc.sync.dma_start(out=pcoord[a][:], in_=pn[a])

            # ---- chunk-0-first index math: the same chain on just the
            # first C columns, so chunk 0's wrap matmuls + gathers can start
            # ~30us before the whole-J chain below completes.
            c0 = slice(0, C)
            t2s = tp.tile([128, C], f32, tag="t0")
            nc.vector.tensor_scalar(
                out=t2s[:], in0=t0[:, c0], scalar1=2.0, scalar2=1.0,
                op0=Alu.mult, op1=Alu.subtract)
            u1s = tp.tile([128, C], f32, tag="t0")
            nc.vector.tensor_scalar(
                out=u1s[:], in0=t2s[:], scalar1=1.0, scalar2=0.5,
                op0=Alu.add, op1=Alu.mult)
            u2s = tp.tile([128, C], f32, tag="t0")
            nc.vector.tensor_scalar_mul(u2s[:], u1s[:], 127.0)
            mxs = tp.tile([128, C], f32, tag="t0")
            nc.vector.tensor_scalar_add(mxs[:], u2s[:], MAGIC)
            fxs = cp.tile([128, C], f32, tag="fxs")
            nc.vector.tensor_scalar_sub(fxs[:], mxs[:], MAGIC)
            q1s = tp.tile([128, C], f32, tag="t0")
            nc.vector.tensor_scalar(
                out=q1s[:], in0=fxs[:], scalar1=0.5, scalar2=-0.25,
                op0=Alu.mult, op1=Alu.add)
            mqs = tp.tile([128, C], f32, tag="t0")
            nc.vector.tensor_scalar_add(mqs[:], q1s[:], MAGIC)
            qrs = cp.tile([128, C], f32, tag="qrs")
            nc.vector.tensor_scalar_sub(qrs[:], mqs[:], MAGIC)
            rowf0s = [None, None, None]
            for a in (2, 1, 0):
                v1s = tp.tile([128, C], f32, tag="t0")
                nc.vector.tensor_scalar(
                    out=v1s[:], in0=pcoord[a][:, c0], scalar1=1.0, scalar2=0.5,
                    op0=Alu.add, op1=Alu.mult)
                v2s = tp.tile([128, C], f32, tag="t0")
                nc.vector.tensor_scalar(
                    out=v2s[:], in0=v1s[:], scalar1=511.0, scalar2=0.0,
                    op0=Alu.mult, op1=Alu.max)
                m3s = tp.tile([128, C], f32, tag="t0")
                nc.vector.tensor_scalar(
                    out=m3s[:], in0=v2s[:], scalar1=511.0, scalar2=MAGIC,
                    op0=Alu.min, op1=Alu.add)
                f64s = tp.tile([128, C], f32, tag="t0")
                nc.vector.tensor_scalar(
                    out=f64s[:], in0=m3s[:], scalar1=MAGIC, scalar2=64.0,
                    op0=Alu.subtract, op1=Alu.mult)
                rowf0 = cp.tile([128, C], f32, tag=f"rowf0_{a}")
                nc.vector.tensor_tensor(
                    out=rowf0[:], in0=f64s[:], in1=qrs[:], op=Alu.add)
                rowf0s[a] = rowf0

            # ---- whole-J index math.
            # time path: fx = rhe(((t+1)*0.5)*127), t = 2*t01-1.  Fused
            # tensor_scalar op pairs are chosen so the result is identical
            # whether or not the intermediate rounds to f32 (2nd op is an
            # exact pow2 scale / exact add / min/max / Sterbenz subtract).
            t2 = tp.tile([128, J], f32, tag="t")
            nc.vector.tensor_scalar(
                out=t2[:], in0=t0[:], scalar1=2.0, scalar2=1.0,
                op0=Alu.mult, op1=Alu.subtract)
            u1 = tp.tile([128, J], f32, tag="t")
            nc.vector.tensor_scalar(
                out=u1[:], in0=t2[:], scalar1=1.0, scalar2=0.5,
                op0=Alu.add, op1=Alu.mult)
            u2 = tp.tile([128, J], f32, tag="t")
            nc.vector.tensor_scalar_mul(u2[:], u1[:], 127.0)
            mx = tp.tile([128, J], f32, tag="t")
            nc.vector.tensor_scalar_add(mx[:], u2[:], MAGIC)
            fx = cp.tile([128, J], f32)
            nc.vector.tensor_scalar_sub(fx[:], mx[:], MAGIC)
            # qr = ix>>1 = rhe(fx*0.5 - 0.25)  (both ops exact)
            q1 = tp.tile([128, J], f32, tag="t")
            nc.vector.tensor_scalar(
                out=q1[:], in0=fx[:], scalar1=0.5, scalar2=-0.25,
                op0=Alu.mult, op1=Alu.add)
            mq = tp.tile([128, J], f32, tag="t")
            nc.vector.tensor_scalar_add(mq[:], q1[:], MAGIC)
            qr = cp.tile([128, J], f32)
            nc.vector.tensor_scalar_sub(qr[:], mq[:], MAGIC)
            rowfs = [None, None, None]
            for a in (2, 1, 0):
                # iy: v = ((p+1)*0.5)*511, clip, rhe; row = iy*64 + qr
                v1 = tp.tile([128, J], f32, tag="t")
                nc.vector.tensor_scalar(
                    out=v1[:], in0=pcoord[a][:], scalar1=1.0, scalar2=0.5,
                    op0=Alu.add, op1=Alu.mult)
                v2 = tp.tile([128, J], f32, tag="t")
                nc.vector.tensor_scalar(
                    out=v2[:], in0=v1[:], scalar1=511.0, scalar2=0.0,
                    op0=Alu.mult, op1=Alu.max)
                m3 = tp.tile([128, J], f32, tag="t")
                nc.vector.tensor_scalar(
                    out=m3[:], in0=v2[:], scalar1=511.0, scalar2=MAGIC,
                    op0=Alu.min, op1=Alu.add)
                f64 = tp.tile([128, J], f32, tag="t")
                nc.vector.tensor_scalar(
                    out=f64[:], in0=m3[:], scalar1=MAGIC, scalar2=64.0,
                    op0=Alu.subtract, op1=Alu.mult)
                rowf = cp.tile([128, J], f32, tag=f"rowf_{a}")
                nc.vector.tensor_tensor(
                    out=rowf[:], in0=f64[:], in1=qr[:], op=Alu.add)
                rowfs[a] = rowf

            # parity bit = fx - 2*qr (exact); off the rowf critical path --
            # first consumer is consume(0), well after the gathers start
            tb = tp.tile([128, J], f32, tag="t")
            nc.vector.tensor_scalar_mul(tb[:], qr[:], 2.0)
            bitf = tp.tile([128, J], f32, tag="t")
            nc.vector.tensor_tensor(
                out=bitf[:], in0=fx[:], in1=tb[:], op=Alu.subtract)
            bit = cp.tile([128, J], i32)
            nc.scalar.copy(out=bit[:], in_=bitf[:])

            def consume(k, gs, halves=1):
                for h in range(halves):
                    hs = slice(h * (C // halves), (h + 1) * (C // halves))
                    sl = slice(k * C + h * (C // halves),
                               k * C + (h + 1) * (C // halves))
                    consume_part(sl, hs, gs)

            def consume_part(sl, hs, gs):
                # product on 64-wide pairs, then select the 128B half by the
                # shared ix-parity bit (in place); compact on the Activation
                # engine (dedicated SBUF ports -- keeps the shared DVE/GpSimd
                # port pair free for the gather ucode) so the store is one
                # contiguous DMA
                n = sl.stop - sl.start
                nc.vector.tensor_tensor(
                    out=gs[0][:, hs, :], in0=gs[0][:, hs, :],
                    in1=gs[1][:, hs, :], op=Alu.mult)
                nc.vector.tensor_tensor(
                    out=gs[0][:, hs, :], in0=gs[0][:, hs, :],
                    in1=gs[2][:, hs, :], op=Alu.mult)
                pred = bit[:, sl][:, :, None].to_broadcast([128, n, FEAT])
                nc.vector.copy_predicated(
                    out=gs[0][:, hs, 0:FEAT], mask=pred,
                    data=gs[0][:, hs, FEAT:2 * FEAT])
                fc = fp.tile([128, C, FEAT], bf16, tag="fc")
                nc.scalar.copy(out=fc[:, 0:n, :], in_=gs[0][:, hs, 0:FEAT])
                nc.sync.dma_start(out=feat[:, sl, :], in_=fc[:, 0:n, :])

            pending = None
            for k in range(NCHUNK):
                sl = slice(k * C, (k + 1) * C)
                gs = []
                for a in range(3):
                    # fold [128,C] row indices into wrapped [128, 8C] int16:
                    # 8 selection matmuls (psum[:, b, :] = rows b*16..b*16+15
                    # of rowf replicated to all 8 partition groups), then one
                    # strided copy interleaving (b, c) -> slot c*8+b.
                    ps = pp.tile([128, 8, C], f32, tag="ps")
                    rhs = rowf0s[a][:] if k == 0 else rowfs[a][:, sl]
                    for b in range(8):
                        nc.tensor.matmul(
                            out=ps[:, b, :],
                            lhsT=sel[:, b * 128:(b + 1) * 128],
                            rhs=rhs,
                            start=True, stop=True)
                    wrapped = wp.tile([128, 8 * C], i16, tag="w")
                    wr_view = bass.AP(
                        wrapped.tensor, wrapped[:].offset,
                        [wrapped[:].ap[0], (1, 8), (8, C)])
                    # interleave+cast on ACT: its own engine queue + dedicated
                    # SBUF ports, so a product op blocked on a gather-DMA sem
                    # can never starve the idx-tile supply
                    nc.scalar.copy(out=wr_view, in_=ps[:])

                    g = gp.tile([128, C, 2 * FEAT], bf16, tag="g")
                    raw_dma_gather(
                        nc.gpsimd, g[:], tabs[a][:, 0:2 * FEAT], wrapped[:],
                        num_idxs=NIDX, elem_size=2 * FEAT,
                        elem_step=4 * FEAT, queue_num=(k * 3 + a) % 4,
                    )
                    gs.append(g)

                # consume the previous chunk only now: keeps this chunk's
                # gather stream ahead of the product ops in every engine queue
                if pending is not None:
                    consume(k - 1, pending)
                pending = gs
            consume(NCHUNK - 1, pending, halves=2)

    # Tile assigns DMASW completion sems round-robin in *scheduled* order,
    # and the SWDGE ucode requires each DMASW sem to be driven by a single
    # queue. Re-derive queue_num from the assigned sem so sem i belongs to
    # queue i%4 always.
    import re
    for blk in nc.main_func.blocks:
        for ins in blk.instructions:
            if isinstance(ins, mybir.InstDMAGatherAnt) and ins.sync_info:
                for u in ins.sync_info.on_update:
                    m = re.match(r"DMASW(\d+)_", getattr(u, "ant_name", "") or "")
                    if m:
                        ins.queue_num = int(m.group(1)) % 4
    nc.finalize()
    return nc


def _get_nc():
    if "nc" not in _CACHE:
        _CACHE["nc"] = _build_nc()
    return _CACHE["nc"]


def _make_sel():
    # sel_in[p, b*128 + p'] = 1 iff p == b*16 + (p' % 16)
    sel = np.zeros((128, 8, 128), dtype=np.float32)
    for b in range(8):
        for pp_ in range(128):
            sel[b * 16 + (pp_ % 16), b, pp_] = 1.0
    return sel.reshape(128, 8 * 128)


def _pack_tables(planes):
    # [F,H,W] -> [H*W, F] -> pair view [H*W/2, 2F] bf16, padded to a 256B
    # row stride; row iy*64+(ix>>1), payload in the first 2F columns
    out = []
    for p in planes:
        pairs = np.ascontiguousarray(
            np.asarray(p, dtype=np.float32).transpose(1, 2, 0)
        ).reshape(RES_H * RES_W // 2, 2 * FEAT).astype(ml_dtypes.bfloat16)
        tab = np.zeros((RES_H * RES_W // 2, 4 * FEAT), dtype=ml_dtypes.bfloat16)
        tab[:, :2 * FEAT] = pairs
        out.append(tab)
    return out


def _host_feat_row(prow, trow, planes):
    """Exact f32 replication of the reference gather/product for one point."""
    one = np.float32(1.0)
    half = np.float32(0.5)
    acc = np.float32(1.0)
    for a, plane in enumerate(planes):
        u = ((trow + one) * half) * np.float32(RES_W - 1)
        ix = int(np.clip(np.round(u).astype(np.int32), 0, RES_W - 1))
        v = ((prow[a] + one) * half) * np.float32(RES_H - 1)
        iy = int(np.clip(np.round(v).astype(np.int32), 0, RES_H - 1))
        acc = (acc * plane[:, iy, ix].astype(np.float32)).astype(np.float32)
    return acc


def _make_in_maps(pnorm, t01, planes):
    pn_pad = np.zeros((NPAD, 3), dtype=np.float32)
    pn_pad[:N] = pnorm
    t_pad = np.zeros(NPAD, dtype=np.float32)
    t_pad[:N] = t01

    tabs = _pack_tables(planes)
    sel = _make_sel()

    in_maps = []
    for c in range(NCORES):
        s = slice(c * NPC, (c + 1) * NPC)
        # device point order is partition-minor: point i -> (i%128, i//128);
        # coords per-plane contiguous [3, 128, J]
        pn_dev = np.ascontiguousarray(
            pn_pad[s].reshape(J, 128, 3).transpose(2, 1, 0))
        t_dev = np.ascontiguousarray(t_pad[s].reshape(J, 128).T)
        in_maps.append({
            "pnorm": pn_dev,
            "time_in": t_dev,
            "sel_in": sel,
            "tab0": tabs[0],
            "tab1": tabs[1],
            "tab2": tabs[2],
        })
    return in_maps


def _device_feat(pnorm, t01, planes, trace=False, **kw):
    """Run the 8-core device kernel; returns (feat[:N], BassKernelResults)."""
    from concourse.bass_utils import run_bass_kernel_spmd

    in_maps = _make_in_maps(pnorm, t01, planes)
    nc = _get_nc()
    res = run_bass_kernel_spmd(nc, in_maps, list(range(NCORES)), trace=trace, **kw)
    feat = np.empty((NPAD, FEAT), dtype=np.float32)
    for c in range(NCORES):
        # undo partition-minor order
        feat[c * NPC:(c + 1) * NPC] = (
            res.results[c]["feat"].transpose(1, 0, 2).reshape(NPC, FEAT)
            .astype(np.float32))
    return feat[:N], res


def kernel(pts, time, plane0, plane1, plane2):
    pts = np.asarray(pts, dtype=np.float32)
    time = np.asarray(time, dtype=np.float32)
    planes = tuple(np.asarray(p, dtype=np.float32) for p in (plane0, plane1, plane2))

    # host: exact f32 normalization (single IEEE divide, matches XLA bitwise)
    pnorm = np.divide(np.negative(pts), np.float32(BOUNDS), dtype=np.float32)
    t01 = time[:, 0]

    feat_orig, _ = _device_feat(pnorm, t01, planes)

    # host fix-up for the reference's last-row shift quirk (exact f32)
    ts32 = np.float32(TIME_STEP)
    p_last = pnorm[-1].copy()
    t_last = np.float32(time[-1, 0] * np.float32(2.0) - np.float32(1.0))
    p_shift = (p_last - ts32).astype(np.float32)
    t_shift = np.float32(t_last - ts32)
    shift_row = _host_feat_row(p_shift, t_shift, planes)

    cond = bool(p_last[0] + ts32 > np.float32(1.0))

    feature_A = feat_orig
    feature_B = feat_orig.copy()
    if cond:
        feature_A = feature_A.copy()
        feature_A[-1] = shift_row
    else:
        feature_B[-1] = shift_row
    return feature_A, feature_B

